# revision 1
# baseline (speedup 1.0000x reference)
"""Trainium2 Bass kernel for nn_DecoderBlock (B=32, T=512, D=512, H=8, FFN=2048).

Sharding: data-parallel over batch, 4 batch elements per core across 8 cores.
On-chip layout: activations are feature-major (X.T = [d, t]); all weights are
host-pre-transposed so every matmul operand is a plain contiguous tile.
Softmax runs without max-subtraction (scores are O(1) scale); the self-attn
mask is added on the PE (identity-matmul accumulate of a host-precomputed
additive mask, pre-scaled by 8 so the ACT exp's 1/8 scale recovers -1e10);
the cross-attn mask is a per-partition ACT bias. Softmax denominators come
free from a ones-column appended to V. LayerNorm stats use ones-matmuls and
the scale/shift is applied via PE outer-products (gamma x rstd, gamma x
mu*rstd - beta). Matmuls run in float32r (full-speed fp32 mode, free dim >=
256); every producer feeding a matmul writes through a float32r view.
"""
import sys

sys.path.insert(0, '/opt/trn_rl_repo')

import numpy as np

D = 512
T = 512
H = 8
DH = 64
FFN = 2048
B = 32
NCORES = 8
NB = B // NCORES  # batch elements per core
P = 128
NDC = D // P     # 4 feature chunks
NHC = FFN // P   # 16 ffn-hidden chunks
NEG = -1.0e10
EPS = 1e-5

_BUILD_CACHE = {}


def build(nb=NB, reps=1, loop_n=0, act_lrelu=True, dma_lite=False, skip=()):
    key = (nb, reps, loop_n, act_lrelu, dma_lite, tuple(skip))
    if key in _BUILD_CACHE:
        return _BUILD_CACHE[key]

    import concourse.bass as bass  # noqa: F401
    import concourse.tile as tile
    import concourse.mybir as mybir
    from concourse import bacc
    from concourse.alu_op_type import AluOpType
    from concourse.masks import make_identity
    from contextlib import ExitStack

    F32 = mybir.dt.float32
    F32R = mybir.dt.float32r
    AF = mybir.ActivationFunctionType

    def r(ap):
        return ap.bitcast(F32R)

    nc = bacc.Bacc()

    # ---- DRAM I/O ----
    xt = nc.dram_tensor("xt", [nb, D, T], F32R, kind="ExternalInput")
    et = nc.dram_tensor("et", [nb, D, T], F32R, kind="ExternalInput")
    maskt = nc.dram_tensor("maskt", [nb, T, T], F32R, kind="ExternalInput")
    ebias = nc.dram_tensor("ebias", [nb, P, NDC], F32, kind="ExternalInput")
    wname = ["wqt1", "wkt1", "wvt1", "wot1", "wqt2", "wkt2", "wvt2", "wot2"]
    wdr = {n: nc.dram_tensor(n, [D, D], F32R, kind="ExternalInput") for n in wname}
    w1t = nc.dram_tensor("w1t", [D, FFN], F32R, kind="ExternalInput")
    w2t = nc.dram_tensor("w2t", [FFN, D], F32R, kind="ExternalInput")
    b1c = nc.dram_tensor("b1c", [P, NHC], F32, kind="ExternalInput")
    b2c = nc.dram_tensor("b2c", [P, NDC], F32, kind="ExternalInput")
    gb2 = {j: nc.dram_tensor(f"gb2_{j}", [2, D], F32R, kind="ExternalInput")
           for j in (1, 2, 3)}
    ot = nc.dram_tensor("ot", [nb, D, T], F32, kind="ExternalOutput")

    with tile.TileContext(nc) as tc:
        with ExitStack() as ctx:
            ctx.enter_context(nc.allow_low_precision(
                reason="fp32r is fp32-width; rounding only trims low mantissa bits"))
            singles = ctx.enter_context(tc.tile_pool(name="singles", bufs=1))
            wa = ctx.enter_context(tc.tile_pool(name="wa", bufs=8))
            w1p = ctx.enter_context(tc.tile_pool(name="w1", bufs=4))
            w2p = ctx.enter_context(tc.tile_pool(name="w2", bufs=3))
            xtp = ctx.enter_context(tc.tile_pool(name="xt", bufs=4))
            etp = ctx.enter_context(tc.tile_pool(name="et", bufs=4))
            mtp = ctx.enter_context(tc.tile_pool(name="mt", bufs=4))
            qtp = ctx.enter_context(tc.tile_pool(name="qt", bufs=4))
            ktp = ctx.enter_context(tc.tile_pool(name="kt", bufs=4))
            vop = ctx.enter_context(tc.tile_pool(name="vo", bufs=4))
            exp_pool = ctx.enter_context(tc.tile_pool(name="ex", bufs=6))
            otp = ctx.enter_context(tc.tile_pool(name="otl", bufs=4))
            prelnp = ctx.enter_context(tc.tile_pool(name="preln", bufs=4))
            postlnp = ctx.enter_context(tc.tile_pool(name="postln", bufs=8))
            htp = ctx.enter_context(tc.tile_pool(name="ht", bufs=4))
            smp = ctx.enter_context(tc.tile_pool(name="sm", bufs=4))
            r65p = ctx.enter_context(tc.tile_pool(name="r65", bufs=2))
            stgp = ctx.enter_context(tc.tile_pool(name="stg", bufs=2))
            sqp = ctx.enter_context(tc.tile_pool(name="sq", bufs=2))
            psS = ctx.enter_context(tc.tile_pool(name="psS", bufs=2, space="PSUM"))
            psB = ctx.enter_context(tc.tile_pool(name="psB", bufs=4, space="PSUM"))

            # persistent constants (memset can't write fp32r; stage + rounded copy)
            ones_stage = singles.tile([P, T], F32, tag="ones_stage")
            nc.vector.memset(ones_stage, 1.0)
            ones128 = singles.tile([P, 1], F32, tag="ones128")
            nc.vector.tensor_copy(out=r(ones128), in_=ones_stage[:, 0:1])
            ones65 = singles.tile([65, P], F32, tag="ones65")
            nc.vector.tensor_copy(out=r(ones65), in_=ones_stage[0:65, 0:P])
            eps_t = singles.tile([P, 1], F32, tag="eps")
            nc.vector.memset(eps_t, EPS)
            ident_stage = singles.tile([P, P], F32, tag="ident_stage")
            make_identity(nc, ident_stage)
            ident = singles.tile([P, P], F32, tag="ident")
            nc.vector.tensor_copy(out=r(ident), in_=ident_stage)
            tb1 = singles.tile([P, NHC], F32, tag="b1")
            nc.sync.dma_start(out=tb1, in_=b1c.ap())
            tb2 = singles.tile([P, NDC], F32, tag="b2")
            nc.sync.dma_start(out=tb2, in_=b2c.ap())
            tgb = {}
            for j in (1, 2, 3):
                tgb[j] = singles.tile([2, D], F32, tag=f"gb{j}", name=f"gb{j}t")
                nc.sync.dma_start(out=r(tgb[j]), in_=gb2[j].ap())
            # rhs2: row0 = mu*rstd (rewritten per LN), row1 = -1 (constant).
            # Init both rows at base 0 (engines can't address base partition 1).
            rhs2_stage = singles.tile([2, T], F32, tag="rhs2_stage")
            nc.vector.memset(rhs2_stage, -1.0)
            rhs2 = singles.tile([2, T], F32, tag="rhs2")
            nc.vector.tensor_copy(out=r(rhs2), in_=rhs2_stage)

            def load_w_tiles(name):
                # dma_lite: TIMING EXPERIMENT ONLY — fetch 1/8 of each weight
                # tile (same instruction count / compute, 1/8 DMA bytes).
                w = D // 8 if dma_lite else D
                tiles = []
                for c in range(NDC):
                    t = wa.tile([P, D], F32, tag="wa")
                    nc.sync.dma_start(out=r(t[:, 0:w]),
                                      in_=wdr[name].ap()[c * P:(c + 1) * P, 0:w])
                    tiles.append(t)
                return tiles

            def proj_fm(wtiles, src, out_pool, tag):
                """out.T[dout,t] = W @ src.T (evac on ACT to offload DVE)."""
                outs = []
                for dc in range(NDC):
                    ps = psB.tile([P, T], F32, tag="psB")
                    for kc in range(NDC):
                        nc.tensor.matmul(ps, r(wtiles[kc][:, dc * P:(dc + 1) * P]),
                                         r(src[kc]), start=(kc == 0), stop=(kc == NDC - 1))
                    o = out_pool.tile([P, T], F32, tag=tag)
                    nc.scalar.activation(out=r(o), in_=ps, func=AF.Copy)
                    outs.append(o)
                return outs

            def proj_vones(wvtiles, src):
                """Token-major V with ones columns: vo[kc] = [128(k), 8*65]."""
                vos = []
                for kc in range(NDC):
                    ps = psB.tile([P, T], F32, tag="psB")
                    for dcd in range(NDC):
                        nc.tensor.matmul(ps, r(src[dcd][:, kc * P:(kc + 1) * P]),
                                         r(wvtiles[dcd]), start=(dcd == 0), stop=(dcd == NDC - 1))
                    vo = vop.tile([P, H * 65], F32, tag="vo")
                    nc.vector.tensor_copy(
                        out=r(vo.rearrange("p (h c) -> p h c", c=65)[:, :, 64:65]),
                        in_=ones_stage[:, 0:H].rearrange("p (h c) -> p h c", c=1))
                    nc.vector.tensor_copy(
                        out=r(vo.rearrange("p (h c) -> p h c", c=65)[:, :, 0:64]),
                        in_=ps.rearrange("p (h c) -> p h c", c=64))
                    vos.append(vo)
                return vos

            def attention(qt, kt, vo, is_self, mts=None, ebias_t=None):
                """Multi-head attention; returns 4 OT tiles [128, T] (feature-major)."""
                ot_tiles = [otp.tile([P, T], F32, tag="otl", name=f"otl{i}")
                            for i in range(NDC)]
                e_tiles = {}

                def scores_exp(h):
                    base = (h % 2) * DH
                    cb = h // 2
                    es = []
                    for pair in range(2):
                        sp = psS.tile([P, 2 * T], F32, tag="psS")
                        for half in range(2):
                            kc = pair * 2 + half
                            sl = sp[:, half * T:(half + 1) * T]
                            nc.tensor.matmul(sl,
                                             r(kt[cb][base:base + DH, kc * P:(kc + 1) * P]),
                                             r(qt[cb][base:base + DH, :]),
                                             start=True, stop=is_self is False)
                            if is_self:
                                # += 8*mask (exp scale 1/8 recovers -1e10)
                                nc.tensor.matmul(sl, r(ident), r(mts[kc]),
                                                 start=False, stop=True)
                        e = exp_pool.tile([P, 2 * T], F32, tag="ex")
                        if is_self:
                            nc.scalar.activation(out=r(e), in_=sp, func=AF.Exp,
                                                 scale=0.125)
                        else:
                            # same bias for both halves of the pair is wrong;
                            # per-half exp with per-kc bias
                            for half in range(2):
                                kc = pair * 2 + half
                                nc.scalar.activation(
                                    out=r(e[:, half * T:(half + 1) * T]),
                                    in_=sp[:, half * T:(half + 1) * T], func=AF.Exp,
                                    bias=ebias_t[:, kc:kc + 1], scale=0.125)
                        es.append(e)
                    e_tiles[h] = es

                def pv_norm(h):
                    cb = h // 2
                    es = e_tiles.pop(h)
                    pv = psB.tile([65, T], F32, tag="psB")
                    for kc in range(NDC):
                        nc.tensor.matmul(pv, r(vo[kc][:, h * 65:(h + 1) * 65]),
                                         r(es[kc // 2][:, (kc % 2) * T:(kc % 2 + 1) * T]),
                                         start=(kc == 0), stop=(kc == NDC - 1))
                    r65 = r65p.tile([65, T], F32, tag="r65")
                    nc.vector.reciprocal(out=r(r65[64:65, :]), in_=pv[64:65, :])
                    rb = psB.tile([P, T], F32, tag="psB")
                    nc.tensor.matmul(rb[0:DH, :], r(ones65[64:65, 0:DH]),
                                     r(r65[64:65, :]), start=True, stop=True)
                    rbs = stgp.tile([DH, T], F32, tag="rbs")
                    nc.vector.tensor_copy(out=rbs, in_=rb[0:DH, :])
                    if h % 2 == 0:
                        nc.vector.tensor_tensor(out=r(ot_tiles[cb][0:DH, :]),
                                                in0=pv[0:DH, :], in1=rbs,
                                                op=AluOpType.mult)
                    else:
                        stg = stgp.tile([DH, T], F32, tag="stg")
                        nc.vector.tensor_tensor(out=r(stg), in0=pv[0:DH, :],
                                                in1=rbs, op=AluOpType.mult)
                        nc.sync.dma_start(out=r(ot_tiles[cb][DH:P, :]), in_=r(stg))

                prev = None
                for h in range(H):
                    scores_exp(h)
                    if prev is not None:
                        pv_norm(prev)
                    prev = h
                pv_norm(prev)
                return ot_tiles

            def out_proj_residual(wtiles, ot_tiles, resid):
                outs = []
                for dc in range(NDC):
                    ps = psB.tile([P, T], F32, tag="psB")
                    for ic in range(NDC):
                        nc.tensor.matmul(ps, r(wtiles[ic][:, dc * P:(dc + 1) * P]),
                                         r(ot_tiles[ic]), start=(ic == 0), stop=(ic == NDC - 1))
                    o = prelnp.tile([P, T], F32, tag="preln")
                    nc.vector.scalar_tensor_tensor(out=r(o), in0=ps, scalar=1.0,
                                                   in1=resid[dc], op0=AluOpType.mult,
                                                   op1=AluOpType.add)
                    outs.append(o)
                return outs

            def layer_norm(src, j, round_out=True):
                """Feature-major layernorm over partition (d) dim.

                Stats via ones-matmuls; scale/shift via PE outer products:
                out = src * (gamma x rstd) - (gamma x mu*rstd - beta)."""
                if 'ln' in skip:
                    return src
                s1 = psB.tile([1, T], F32, tag="psB")
                s2 = psB.tile([1, T], F32, tag="psB")
                for dc in range(NDC):
                    nc.tensor.matmul(s1, r(ones128), r(src[dc]),
                                     start=(dc == 0), stop=(dc == NDC - 1))
                for dc in range(NDC):
                    sq = sqp.tile([P, T], F32, tag="sq")
                    nc.vector.tensor_tensor(out=r(sq), in0=src[dc], in1=src[dc],
                                            op=AluOpType.mult)
                    nc.tensor.matmul(s2, r(ones128), r(sq),
                                     start=(dc == 0), stop=(dc == NDC - 1))
                s1s = smp.tile([1, T], F32, tag="sm")
                nc.scalar.activation(out=s1s, in_=s1, func=AF.Copy)
                s2s = smp.tile([1, T], F32, tag="sm")
                nc.scalar.activation(out=s2s, in_=s2, func=AF.Copy)
                mu = smp.tile([1, T], F32, tag="sm")
                nc.vector.tensor_scalar(out=mu, in0=s1s, scalar1=1.0 / D,
                                        scalar2=None, op0=AluOpType.mult)
                musq = smp.tile([1, T], F32, tag="sm")
                nc.vector.tensor_tensor(out=musq, in0=mu, in1=mu, op=AluOpType.mult)
                var = smp.tile([1, T], F32, tag="sm")
                nc.vector.scalar_tensor_tensor(out=var, in0=s2s, scalar=1.0 / D,
                                               in1=musq, op0=AluOpType.mult,
                                               op1=AluOpType.subtract)
                sd = smp.tile([1, T], F32, tag="sm")
                nc.scalar.activation(out=sd, in_=var, func=AF.Sqrt,
                                     bias=eps_t[0:1, 0:1])
                rstd = smp.tile([1, T], F32, tag="sm")
                nc.vector.reciprocal(out=r(rstd), in_=sd)
                nc.vector.tensor_tensor(out=r(rhs2[0:1, :]), in0=mu, in1=rstd,
                                        op=AluOpType.mult)
                outs = []
                for dc in range(NDC):
                    grs = psB.tile([P, T], F32, tag="psB")
                    nc.tensor.matmul(grs, r(tgb[j][0:1, dc * P:(dc + 1) * P]),
                                     r(rstd), start=True, stop=True)
                    c2 = psB.tile([P, T], F32, tag="psB")
                    nc.tensor.matmul(c2, r(tgb[j][:, dc * P:(dc + 1) * P]),
                                     r(rhs2), start=True, stop=True)
                    o = postlnp.tile([P, T], F32, tag="postln")
                    ow = r(o) if round_out else o
                    nc.vector.tensor_tensor(out=ow, in0=src[dc], in1=grs,
                                            op=AluOpType.mult)
                    nc.vector.tensor_tensor(out=ow, in0=o, in1=c2,
                                            op=AluOpType.subtract)
                    outs.append(o)
                return outs

            def body():
              for b in [bb for _ in range(reps) for bb in range(nb)]:
                # ---- load per-b inputs ----
                xts = []
                for dc in range(NDC):
                    t = xtp.tile([P, T], F32, tag="xt")
                    nc.sync.dma_start(out=r(t), in_=xt.ap()[b, dc * P:(dc + 1) * P, :])
                    xts.append(t)
                mts = []
                for kc in range(NDC):
                    t = mtp.tile([P, T], F32, tag="mt")
                    nc.sync.dma_start(out=r(t), in_=maskt.ap()[b, kc * P:(kc + 1) * P, :])
                    mts.append(t)
                ebias_t = smp.tile([P, NDC], F32, tag="sm_eb")
                nc.sync.dma_start(out=ebias_t, in_=ebias.ap()[b])

                # ---- self attention ----
                wq = load_w_tiles("wqt1")
                qt = proj_fm(wq, xts, qtp, "qt")
                wk = load_w_tiles("wkt1")
                kt = proj_fm(wk, xts, ktp, "kt")
                wv = load_w_tiles("wvt1")
                vo = proj_vones(wv, xts)
                ot_t = qt if 'attn' in skip else attention(qt, kt, vo, True, mts=mts)
                wo = load_w_tiles("wot1")
                y0 = out_proj_residual(wo, ot_t, xts)
                yt = layer_norm(y0, 1)

                # ---- cross attention ----
                ets = []
                for dc in range(NDC):
                    t = etp.tile([P, T], F32, tag="et")
                    nc.sync.dma_start(out=r(t), in_=et.ap()[b, dc * P:(dc + 1) * P, :])
                    ets.append(t)
                wq2 = load_w_tiles("wqt2")
                qt2 = proj_fm(wq2, yt, qtp, "qt")
                wk2 = load_w_tiles("wkt2")
                kt2 = proj_fm(wk2, ets, ktp, "kt")
                wv2 = load_w_tiles("wvt2")
                vo2 = proj_vones(wv2, ets)
                ot2 = qt2 if 'attn' in skip else attention(qt2, kt2, vo2, False, ebias_t=ebias_t)
                wo2 = load_w_tiles("wot2")
                z0 = out_proj_residual(wo2, ot2, yt)
                zt = layer_norm(z0, 2)

                # ---- FFN ----
                w1tiles = []
                w1w = FFN // 8 if dma_lite else FFN
                for dc in range(NDC):
                    t = w1p.tile([P, FFN], F32, tag="w1")
                    nc.sync.dma_start(out=r(t[:, 0:w1w]),
                                      in_=w1t.ap()[dc * P:(dc + 1) * P, 0:w1w])
                    w1tiles.append(t)
                fps = [psB.tile([P, T], F32, tag="psB", name=f"fps{i}")
                       for i in range(NDC)]
                h_tiles = {}

                def ffn_h(hc):
                    hp2 = psS.tile([P, 2 * T], F32, tag="psS")
                    hp = hp2[:, 0:T]
                    for dc in range(NDC):
                        nc.tensor.matmul(hp, r(w1tiles[dc][:, hc * P:(hc + 1) * P]),
                                         r(zt[dc]), start=(dc == 0), stop=(dc == NDC - 1))
                    ht = htp.tile([P, T], F32, tag="ht")
                    if act_lrelu:
                        nc.scalar.activation(out=r(ht), in_=hp, func=AF.Lrelu,
                                             bias=tb1[:, hc:hc + 1], scale=1.0,
                                             alpha=0.01)
                    else:
                        nc.vector.tensor_scalar(out=r(ht), in0=hp,
                                                scalar1=tb1[:, hc:hc + 1],
                                                scalar2=None, op0=AluOpType.add)
                        nc.vector.scalar_tensor_tensor(out=r(ht), in0=ht, scalar=0.01,
                                                       in1=ht, op0=AluOpType.mult,
                                                       op1=AluOpType.max)
                    h_tiles[hc] = ht

                def ffn_f(hc):
                    ht = h_tiles.pop(hc)
                    w2tile = w2p.tile([P, D], F32, tag="w2")
                    w2w = D // 8 if dma_lite else D
                    nc.sync.dma_start(out=r(w2tile[:, 0:w2w]),
                                      in_=w2t.ap()[hc * P:(hc + 1) * P, 0:w2w])
                    for dc in range(NDC):
                        nc.tensor.matmul(fps[dc], r(w2tile[:, dc * P:(dc + 1) * P]),
                                         r(ht), start=(hc == 0), stop=(hc == NHC - 1))

                prevh = None
                if 'ffn' not in skip:
                    for hc in range(NHC):
                        ffn_h(hc)
                        if prevh is not None:
                            ffn_f(prevh)
                        prevh = hc
                    ffn_f(prevh)

                out0 = []
                if 'ffn' in skip:
                    out0 = zt
                else:
                    for dc in range(NDC):
                        o = prelnp.tile([P, T], F32, tag="preln")
                        nc.vector.scalar_tensor_tensor(out=r(o), in0=fps[dc],
                                                       scalar=tb2[:, dc:dc + 1], in1=zt[dc],
                                                       op0=AluOpType.add, op1=AluOpType.add)
                        out0.append(o)
                outt = layer_norm(out0, 3, round_out=False)
                for dc in range(NDC):
                    nc.sync.dma_start(out=ot.ap()[b, dc * P:(dc + 1) * P, :],
                                      in_=outt[dc])

            if loop_n > 1:
                with tc.For_i(0, loop_n, 1):
                    body()
            else:
                body()

    nc.compile()
    _BUILD_CACHE[key] = nc
    return nc


def prep_core_inputs(inputs, nb=NB):
    """Host-side prep: transpose weights/activations, build masks, shard over cores."""
    X = np.asarray(inputs["X"], np.float32)
    E = np.asarray(inputs["enc_outputs"], np.float32)
    dv = np.asarray(inputs["dec_valid_lens"])
    ev = np.asarray(inputs["enc_valid_lens"])
    pos = np.arange(T)

    shared = {
        "w1t": np.ascontiguousarray(np.asarray(inputs["W1"], np.float32).T),
        "w2t": np.ascontiguousarray(np.asarray(inputs["W2"], np.float32).T),
        "b1c": np.ascontiguousarray(np.asarray(inputs["b1"], np.float32).reshape(NHC, P).T),
        "b2c": np.ascontiguousarray(np.asarray(inputs["b2"], np.float32).reshape(NDC, P).T),
    }
    for j in (1, 2, 3):
        shared[f"gb2_{j}"] = np.ascontiguousarray(np.stack(
            [np.asarray(inputs[f"g{j}"], np.float32),
             np.asarray(inputs[f"be{j}"], np.float32)], axis=0))
    for n, src in [("wqt1", "Wq1"), ("wkt1", "Wk1"), ("wvt1", "Wv1"), ("wot1", "Wo1"),
                   ("wqt2", "Wq2"), ("wkt2", "Wk2"), ("wvt2", "Wv2"), ("wot2", "Wo2")]:
        shared[n] = np.ascontiguousarray(np.asarray(inputs[src], np.float32).T)

    in_maps = []
    ncores = X.shape[0] // nb
    for c in range(ncores):
        sl = slice(c * nb, (c + 1) * nb)
        xtc = np.ascontiguousarray(X[sl].transpose(0, 2, 1))
        etc = np.ascontiguousarray(E[sl].transpose(0, 2, 1))
        # self mask (pre-scaled by 8): maskt[b][k, q] = 8*NEG where k >= dec_valid[b, q]
        mk = (pos[None, :, None] >= dv[sl][:, None, :]).astype(np.float32) * (8.0 * NEG)
        # cross bias per k: ebias[b, p, kc] for k = kc*128 + p
        eb = (pos[None, :] >= ev[sl][:, None]).astype(np.float32) * NEG
        eb = np.ascontiguousarray(eb.reshape(nb, NDC, P).transpose(0, 2, 1))
        m = {"xt": xtc, "et": etc, "maskt": np.ascontiguousarray(mk), "ebias": eb}
        m.update(shared)
        in_maps.append(m)
    return in_maps


def kernel(**inputs):
    from concourse import bass_utils

    nc = build(NB)
    in_maps = prep_core_inputs(inputs, NB)
    res = bass_utils.run_bass_kernel_spmd(nc, in_maps, core_ids=list(range(NCORES)))
    outs = [r["ot"].transpose(0, 2, 1) for r in res.results]
    return np.ascontiguousarray(np.concatenate(outs, axis=0).astype(np.float32))

